# revision 26
# baseline (speedup 1.0000x reference)
"""ACDA (adaptive conv) Trainium2 kernel — 8-core data-parallel over batch.

Per core: one sample (C=64, H=128, W=128). The image is split into two
64-row halves stacked on the 128 SBUF partitions (partition p<64 -> half A
channel p, p>=64 -> half B channel p-64), so every engine op runs with all
128 lanes active.

The 1x1-conv filter generation runs as ONE 128-deep matmul per tap and
pixel chunk: the lhsT is block-diagonal [[W_k, 0], [0, W_k]] over
(c_in, half) x (c_out, half), so a single 512-column rhs stream produces
g_k for BOTH halves at once (the previous per-half 64-deep quadrant split
streamed every pixel chunk twice — 2x the PE time for the same output).

Host-side prep (inside kernel(), not on device): x is zero-padded, bf16-cast
and laid out per-core as two copies (xpadA with a left pad column, xpadB
column-shifted by one) so that all nine 3x3-tap shifts are 4-byte-aligned
SBUF views; weights are block-diagonal lhsT (c_in|half, k, c_out|half).

Pipeline per 16-row tile (both halves at once), overlapped by the Tile
scheduler; the Activation engine is the pacer (saturated end to end):
  DMA: padded x arrives in 4 row-bands (1 tile + halo each)
  PE:  g_k = [[W_k,0],[0,W_k]]^T @ x  (bf16 in, fp32 PSUM), 4 chunks per k
  ACT: f_k = relu(g_k + b_k)  PSUM -> SBUF bf16  (bias is per-partition)
  DVE: prod = f * patch rows 0:13; GPSIMD rows 13:16 (two independent
       streams sized so both fit under the ACT eviction pace)
  DVE/GP: in-place reduction tree over the 9 taps, same 13/3 row split
  DMA: prod[:,8] tile -> DRAM in bf16 (widened to fp32 on the host)

Alternative emission paths kept behind opts for experiments: arch="fold"
(PSUM accumulation of taps via identity matmuls), stt taps (bias preloaded
into PSUM by a K=1 ones matmul + fused relu-mul scalar_tensor_tensor on
DVE), dve_ev taps (eviction via DVE tensor_scalar), stage probes.
"""

import numpy as np
import ml_dtypes
from contextlib import ExitStack

import concourse.bass as bass
import concourse.tile as tile
from concourse import bacc, mybir
from concourse.bass_utils import run_bass_kernel_spmd

B, C, H, W, K = 8, 64, 128, 128, 3
NCORES = 8
RT = 16           # output rows per tile (per half)
BF16 = mybir.dt.bfloat16
F32 = mybir.dt.float32
RELU = mybir.ActivationFunctionType.Relu
MULT = mybir.AluOpType.mult
ADD = mybir.AluOpType.add
MAX = mybir.AluOpType.max

_CACHE = {}


def _window3(ap, lr0, dj, rt):
    """Overlapping 3-row window AP: [128, 3(di), rt, 128] over a padded-x
    band tile [128, nrows, rowlen], starting at buffer row lr0, col dj."""
    a = ap.copy()
    v = a.ap
    row_stride = v[1][0]
    v[1] = [row_stride, 3]
    v[2] = [row_stride, rt]
    v.append([1, 128])
    a.offset = a.offset + lr0 * row_stride + dj
    return a


def _kernel_body(ctx: ExitStack, tc, out_d, xA_d, xB_d, wT_d, bias_d,
                 biasrow_d, ident_d, nreps=1, opts=None):
    nc = tc.nc
    o = dict(rt=RT, psum_bufs=2, fbufs=2, pbufs=2)
    o.update(opts or {})
    rt = o["rt"]
    nt = 64 // rt
    nch = rt // 4          # 4-row matmul chunks per k-tile (1 PSUM bank each)

    inp = ctx.enter_context(tc.tile_pool(name="inp", bufs=1))
    # dep-free dummy relu so the one-time ACT table load runs at t~0 instead
    # of gating the first real eviction
    scratch = inp.tile([128, 2], F32)
    nc.gpsimd.memset(scratch[:], 0.0)
    nc.scalar.activation(scratch[:, 0:1], scratch[:, 1:2], RELU, bias=0.0)
    # dep-free dummy matmuls so the PE p-state ramps to full clock while the
    # input DMA is still in flight (HAM needs ~3us of sustained PE activity)
    warm = inp.tile([128, 512], BF16)
    nc.vector.memset(warm[:], 0.0)
    # lhsT per k: block-diagonal [[W_k, 0], [0, W_k]] so one 128-contraction
    # matmul computes tap k for BOTH image halves in a single rhs stream
    # (the 64-deep quadrant split would stream every pixel chunk twice)
    wT = inp.tile([128, 9, 128], BF16)
    bias = inp.tile([128, 9], F32)
    # x staged in 4 row-bands (16 rows + halo each) so the first matmuls can
    # start after ~1/4 of the input DMA. Band 0 is issued before the weights:
    # its transfer is the long pole for the first eviction.
    xAb = [inp.tile([128, 18, 130], BF16, name=f"xAb{b}", tag=f"xA{b}")
           for b in range(4)]
    xBb = [inp.tile([128, 18, 128], BF16, name=f"xBb{b}", tag=f"xB{b}")
           for b in range(4)]
    nc.sync.dma_start(wT[:], wT_d[:])
    nc.sync.dma_start(xAb[0][:], xA_d[:, 0:18, :])
    nc.sync.dma_start(bias[:], bias_d[:])
    nc.sync.dma_start(xBb[0][:], xB_d[:, 0:18, :])
    for b in range(1, 4):
        nc.sync.dma_start(xAb[b][:], xA_d[:, 16 * b: 16 * b + 18, :])
        nc.sync.dma_start(xBb[b][:], xB_d[:, 16 * b: 16 * b + 18, :])

    arch = o.get("arch", "tree")
    psum = ctx.enter_context(tc.tile_pool(name="psum", bufs=o["psum_bufs"],
                                          space="PSUM"))
    fpool = ctx.enter_context(tc.tile_pool(name="f", bufs=o["fbufs"]))
    ppool = ctx.enter_context(tc.tile_pool(name="prod", bufs=o["pbufs"]))
    if arch == "fold":
        # out accumulates in PSUM via identity matmuls: no add tree at all
        accp = ctx.enter_context(tc.tile_pool(name="acc",
                                              bufs=o.get("accbufs", 1),
                                              space="PSUM"))
        opool = ctx.enter_context(tc.tile_pool(name="osb", bufs=2))
        ident = inp.tile([128, 128], BF16)
        nc.sync.dma_start(ident[:], ident_d[:])

    if o.get("warm", 8):
        wps = psum.tile([128, rt, 128], F32, name="warm_ps", tag="ps")
        for i in range(o.get("warm", 8)):
            nc.tensor.matmul(wps[0:64, 0:4, :], warm[0:64, 0:64],
                             warm[0:64].rearrange("p (a b) -> p a b", a=4),
                             start=True, stop=True)

    out4 = out_d.rearrange("c (h r) w -> h c r w", h=2)

    def _taps(v, default):
        if v is None:
            return default
        if isinstance(v, int):
            return (v,)
        if isinstance(v, str):
            return tuple(int(c) for c in v if c.isdigit())
        return tuple(v)

    stt_taps = _taps(o.get("stt"), ())
    dve_ev = _taps(o.get("dve_ev"), ())
    ms = o.get("msplit", 13)       # DVE rows per mul op; Pool gets the rest
    asp = o.get("asplit", 13)      # DVE rows per add op; Pool gets the rest
    if stt_taps:
        ones = inp.tile([1, 4, 128], BF16)
        nc.vector.memset(ones[:], 1.0)
        biasrow = inp.tile([1, 9, 128], BF16)
        nc.sync.dma_start(biasrow[:], biasrow_d[:])

    for t in range(nt * nreps):
        r0 = (t % nt) * rt
        band = r0 // 16
        lr0 = r0 - 16 * band
        xA, xB = xAb[band], xBb[band]
        f = fpool.tile([128, 9, rt, 128], BF16)
        prod = ppool.tile([128, 9, rt, 128], BF16)
        pss = {}

        def patch_ap(k, r_a, r_b):
            di, dj = divmod(k, 3)
            if dj == 1:
                return xB[:, lr0 + di + r_a: lr0 + di + r_b, 0:128]
            return xA[:, lr0 + di + r_a: lr0 + di + r_b, dj:dj + 128]

        def kgroup(k):
            ps = psum.tile([128, rt, 128], F32)
            pss[k] = ps
            if o.get("stage") != "nomm":
                fused_bias = k in stt_taps
                if fused_bias:
                    # bias arrives in PSUM via a K=1 ones matmul so the
                    # STT eviction-mul can fuse relu+mul without a bias op
                    for n in range(nch):
                        nc.tensor.matmul(ps[:, 4 * n: 4 * n + 4, :],
                                         biasrow[0:1, k, :], ones[0:1],
                                         start=True, stop=False)
                # center pixels: buffer row r+1, buffer cols 1..128
                for n in range(nch):  # <=512 fp32 cols per matmul (1 bank)
                    rhs = xA[:, lr0 + 1 + 4 * n: lr0 + 5 + 4 * n, 1:129]
                    nc.tensor.matmul(
                        ps[:, 4 * n: 4 * n + 4, :],
                        wT[:, k, :],
                        rhs,
                        start=not fused_bias, stop=True,
                    )
            if o.get("stage") == "noact" or k in stt_taps:
                return
            if k in dve_ev:
                nc.vector.tensor_scalar(f[:, k], ps[:], bias[:, k:k + 1], 0.0,
                                        op0=ADD, op1=MAX)
            else:
                nc.scalar.activation(f[:, k], ps[:], RELU, bias=bias[:, k:k + 1])

        def mul(k):
            if k in stt_taps:
                # fused relu+mul straight from PSUM (bias already inside)
                nc.vector.scalar_tensor_tensor(prod[:, k], pss[k][:], 0.0,
                                               patch_ap(k, 0, rt),
                                               op0=MAX, op1=MULT)
                return
            nc.vector.tensor_tensor(prod[:, k, 0:ms], f[:, k, 0:ms],
                                    patch_ap(k, 0, ms), op=MULT)
            if ms < rt:
                nc.gpsimd.tensor_tensor(prod[:, k, ms:rt], f[:, k, ms:rt],
                                        patch_ap(k, ms, rt), op=MULT)

        def add(dst, src):
            nc.vector.tensor_tensor(prod[:, dst, 0:asp], prod[:, src, 0:asp],
                                    prod[:, dst, 0:asp], op=ADD)
            if asp < rt:
                nc.gpsimd.tensor_tensor(prod[:, dst, asp:rt],
                                        prod[:, src, asp:rt],
                                        prod[:, dst, asp:rt], op=ADD)

        stage = o.get("stage", "full")
        if stage == "nomm":
            # pure ACT pace probe: all evictions read one pre-filled psum tile
            if t == 0:
                pse = psum.tile([128, rt, 128], F32, tag="ps")
                for n in range(nch):
                    nc.tensor.matmul(pse[:, 4 * n: 4 * n + 4, :],
                                     wT[:, 0, :],
                                     xA[:, 1 + 4 * n: 5 + 4 * n, 1:129],
                                     start=True, stop=True)
                o["_pse"] = pse
            for k in range(9):
                nc.scalar.activation(f[:, k], o["_pse"][:], RELU,
                                     bias=bias[:, k:k + 1])
            nc.sync.dma_start(out4[:, :, r0:r0 + rt, :], f[:, 8])
            continue
        if stage == "noact":
            # pure PE pace probe: matmuls only, DMA from the input band
            for k in range(9):
                kgroup(k)
            nc.sync.dma_start(out_d[:, r0:r0 + rt, :],
                              xA[0:64, 1:1 + rt, 1:129])
            continue
        if stage == "empty":
            # loop-overhead floor: input DMAs + output DMAs only
            nc.sync.dma_start(out_d[:, r0:r0 + rt, :],
                              xA[0:64, 1:1 + rt, 1:129])
            continue
        if stage == "evict":
            for k in range(9):
                kgroup(k)
            nc.sync.dma_start(out4[:, :, r0:r0 + rt, :], f[:, 8])
            continue
        if stage == "noadd":
            for k in range(9):
                kgroup(k)
            for k in range(9):
                mul(k)
            nc.sync.dma_start(out4[:, :, r0:r0 + rt, :], prod[:, 8])
            continue

        if arch == "fold":
            # every tap folds into a PSUM accumulator via identity matmuls;
            # no add tree, final copy PSUM -> SBUF bf16 on ACT, then DMA
            acc = accp.tile([128, rt, 128], F32)
            outsb = opool.tile([128, rt, 128], BF16)
            kseq = o.get("kseq", [0, 3, 1, 5, 2, 7, 4, 6, 8])
            fd = o.get("fdelay", 2)   # folds trail by fd taps so the PE's
                                      # in-order queue never head-of-line
                                      # blocks on the ACT->DVE chain

            def fold(j, k):
                for n in range(nch):
                    nc.tensor.matmul(
                        acc[:, 4 * n: 4 * n + 4, :], ident[:],
                        prod[:, k, 4 * n: 4 * n + 4],
                        start=(j == 0), stop=(j == len(kseq) - 1),
                    )

            for j, k in enumerate(kseq):
                kgroup(k)
                mul(k)
                if j >= fd:
                    fold(j - fd, kseq[j - fd])
            for j in range(len(kseq) - fd, len(kseq)):
                fold(j, kseq[j])
            nc.scalar.activation(outsb[:], acc[:],
                                 mybir.ActivationFunctionType.Copy)
            nc.sync.dma_start(out4[:, :, r0:r0 + rt, :], outsb[:])
            continue

        for k in range(8):
            kgroup(k)
        mul(0)
        mul(1)
        add(1, 0)
        mul(2)
        mul(3)
        add(3, 2)
        add(3, 1)
        mul(4)
        mul(5)
        add(5, 4)
        mul(6)
        mul(7)
        add(7, 6)
        add(7, 5)
        add(7, 3)

        kgroup(8)
        mul(8)
        if o.get("tail2", 0):
            # split the final add so the out DMA starts after the first half
            nc.vector.tensor_tensor(prod[:, 8, 0:8], prod[:, 7, 0:8],
                                    prod[:, 8, 0:8], op=ADD)
            nc.sync.dma_start(out4[:, :, r0:r0 + 8, :], prod[:, 8, 0:8])
            nc.vector.tensor_tensor(prod[:, 8, 8:asp], prod[:, 7, 8:asp],
                                    prod[:, 8, 8:asp], op=ADD)
            if asp < rt:
                nc.gpsimd.tensor_tensor(prod[:, 8, asp:rt], prod[:, 7, asp:rt],
                                        prod[:, 8, asp:rt], op=ADD)
            nc.sync.dma_start(out4[:, :, r0 + 8:r0 + rt, :], prod[:, 8, 8:rt])
        else:
            add(8, 7)
            nc.sync.dma_start(out4[:, :, r0:r0 + rt, :], prod[:, 8])


def _declare_tensors(nc):
    xA_d = nc.dram_tensor("xpadA", (128, 66, 130), BF16, kind="ExternalInput").ap()
    xB_d = nc.dram_tensor("xpadB", (128, 66, 128), BF16, kind="ExternalInput").ap()
    wT_d = nc.dram_tensor("wT", (128, 9, 128), BF16, kind="ExternalInput").ap()
    bias_d = nc.dram_tensor("bias", (128, 9), F32, kind="ExternalInput").ap()
    biasrow_d = nc.dram_tensor("biasrow", (1, 9, 128), BF16,
                               kind="ExternalInput").ap()
    ident_d = nc.dram_tensor("ident", (128, 128), BF16,
                             kind="ExternalInput").ap()
    out_d = nc.dram_tensor("out", (C, H, W), BF16, kind="ExternalOutput").ap()
    return out_d, xA_d, xB_d, wT_d, bias_d, biasrow_d, ident_d


def _build():
    if "nc" in _CACHE:
        return _CACHE["nc"]
    nc = bacc.Bacc("TRN2", target_bir_lowering=False, debug=False,
                   num_devices=NCORES)
    aps = _declare_tensors(nc)
    with tile.TileContext(nc) as tc, ExitStack() as ctx:
        _kernel_body(ctx, tc, *aps)
    nc.compile()
    _CACHE["nc"] = nc
    return nc


def _prep_core_inputs(x_i: np.ndarray, wT_np, bias_np, biasrow_np):
    """x_i: (C, H, W) float32 -> per-core input dict."""
    bf = ml_dtypes.bfloat16
    xA = np.zeros((128, 66, 130), dtype=bf)
    xB = np.zeros((128, 66, 128), dtype=bf)
    xb = x_i.astype(bf)
    # half A: buffer rows 0..65 = x rows -1..64 (row -1 zero-padded)
    xA[0:64, 1:66, 1:129] = xb[:, 0:65, :]
    xB[0:64, 1:66, :] = xb[:, 0:65, :]
    # half B: buffer rows 0..65 = x rows 63..128 (row 128 zero-padded)
    xA[64:128, 0:65, 1:129] = xb[:, 63:128, :]
    xB[64:128, 0:65, :] = xb[:, 63:128, :]
    return {"xpadA": xA, "xpadB": xB, "wT": wT_np, "bias": bias_np,
            "biasrow": biasrow_np, "ident": np.eye(128, dtype=bf)}


def _prep_inputs(x, W_gen, b_gen):
    x = np.asarray(x, dtype=np.float32)
    W_gen = np.asarray(W_gen, dtype=np.float32)
    b_gen = np.asarray(b_gen, dtype=np.float32)

    bf = ml_dtypes.bfloat16
    # lhsT: (c_in, k, c_out); o index in reference = c_out*9 + k.
    # Block-diagonal on (cin, cout) so one matmul serves both image halves.
    wT_half = W_gen.reshape(C, K * K, C).transpose(2, 1, 0).astype(bf)  # (cin,k,cout)
    wT_np = np.zeros((128, K * K, 128), dtype=bf)
    wT_np[0:C, :, 0:C] = wT_half
    wT_np[C:128, :, C:128] = wT_half
    b2 = b_gen.reshape(C, K * K).astype(np.float32)                     # (c_out, k)
    bias_np = np.ascontiguousarray(np.concatenate([b2, b2], axis=0))    # (128, 9)
    # per-k bias over the 128 (c, half) output channels, as a K=1 lhsT row
    biasrow_np = np.ascontiguousarray(
        bias_np.T.reshape(1, K * K, 128)).astype(bf)                    # (1, 9, 128)

    return [_prep_core_inputs(x[i], wT_np, bias_np, biasrow_np)
            for i in range(x.shape[0])]


def kernel(x: np.ndarray, W_gen: np.ndarray, b_gen: np.ndarray) -> np.ndarray:
    nc = _build()
    in_maps = _prep_inputs(x, W_gen, b_gen)
    res = run_bass_kernel_spmd(nc, in_maps, core_ids=list(range(NCORES)))
    out = np.stack([res.results[i]["out"] for i in range(NCORES)], axis=0)
    return out.astype(np.float32)


if __name__ == "__main__":
    xs = np.random.randn(B, C, H, W).astype(np.float32)
    Wg = np.random.randn(C * K * K, C).astype(np.float32) / np.sqrt(C)
    bg = (np.random.randn(C * K * K) * 0.01).astype(np.float32)
    o = kernel(xs, Wg, bg)
    print("out", o.shape, o.dtype, float(np.abs(o).mean()))



# revision 37
# speedup vs baseline: 1.1925x; 1.1925x over previous
"""ACDA (adaptive conv) Trainium2 kernel — 8-core data-parallel over batch.

Per core: one sample (C=64, H=128, W=128). The image is split into two
64-row halves stacked on the 128 SBUF partitions (partition p<64 -> half A
channel p, p>=64 -> half B channel p-64), so every engine op runs with all
128 lanes active.

The 1x1-conv filter generation runs as ONE 128-deep matmul per tap and
pixel chunk: the lhsT is block-diagonal [[W_k, 0], [0, W_k]] over
(c_in, half) x (c_out, half), so a single 512-column rhs stream produces
g_k for BOTH halves at once (the previous per-half 64-deep quadrant split
streamed every pixel chunk twice — 2x the PE time for the same output).

Host-side prep (inside kernel(), not on device): x is zero-padded, bf16-cast
and laid out per-core as two copies (xpadA with a left pad column, xpadB
column-shifted by one) so that all nine 3x3-tap shifts are 4-byte-aligned
SBUF views; weights are block-diagonal lhsT (c_in|half, k, c_out|half).

Pipeline per 16-row tile (both halves at once), overlapped by the Tile
scheduler; the Activation engine is the pacer (saturated end to end):
  DMA: padded x arrives in 4 row-bands (1 tile + halo each)
  PE:  g_k = [[W_k,0],[0,W_k]]^T @ x  (bf16 in, fp32 PSUM), 4 chunks per k
  ACT: f_k = relu(g_k + b_k)  PSUM -> SBUF bf16  (bias is per-partition)
  DVE: prod = f * patch rows 0:13; GPSIMD rows 13:16 (two independent
       streams sized so both fit under the ACT eviction pace)
  DVE/GP: in-place reduction tree over the 9 taps, same 13/3 row split
  DMA: prod[:,8] tile -> DRAM in bf16 (widened to fp32 on the host)

Alternative emission paths kept behind opts for experiments: arch="fold"
(PSUM accumulation of taps via identity matmuls), stt taps (bias preloaded
into PSUM by a K=1 ones matmul + fused relu-mul scalar_tensor_tensor on
DVE), dve_ev taps (eviction via DVE tensor_scalar), stage probes.
"""

import numpy as np
import ml_dtypes
from contextlib import ExitStack

import concourse.bass as bass
import concourse.tile as tile
from concourse import bacc, mybir
from concourse.bass_utils import run_bass_kernel_spmd

B, C, H, W, K = 8, 64, 128, 128, 3
NCORES = 8
RT = 16           # output rows per tile (per half)
BF16 = mybir.dt.bfloat16
F32 = mybir.dt.float32
RELU = mybir.ActivationFunctionType.Relu
MULT = mybir.AluOpType.mult
ADD = mybir.AluOpType.add
MAX = mybir.AluOpType.max

_CACHE = {}


def _window3(ap, lr0, dj, rt):
    """Overlapping 3-row window AP: [128, 3(di), rt, 128] over a padded-x
    band tile [128, nrows, rowlen], starting at buffer row lr0, col dj."""
    a = ap.copy()
    v = a.ap
    row_stride = v[1][0]
    v[1] = [row_stride, 3]
    v[2] = [row_stride, rt]
    v.append([1, 128])
    a.offset = a.offset + lr0 * row_stride + dj
    return a


def _kernel_body(ctx: ExitStack, tc, out_d, xA_d, xB_d, wT_d, bias_d,
                 biasrow_d, ident_d, nreps=1, opts=None):
    nc = tc.nc
    o = dict(rt=RT, psum_bufs=2, fbufs=2, pbufs=2)
    o.update(opts or {})
    rt = o["rt"]
    nt = 64 // rt
    nch = rt // 4          # 4-row matmul chunks per k-tile (1 PSUM bank each)

    inp = ctx.enter_context(tc.tile_pool(name="inp", bufs=1))
    # dep-free dummy relu so the one-time ACT table load runs at t~0 instead
    # of gating the first real eviction
    scratch = inp.tile([128, 2], F32)
    nc.gpsimd.memset(scratch[:], 0.0)
    nc.scalar.activation(scratch[:, 0:1], scratch[:, 1:2], RELU, bias=0.0)
    # dep-free dummy matmuls so the PE p-state ramps to full clock while the
    # input DMA is still in flight (HAM needs ~3us of sustained PE activity)
    warm = inp.tile([128, 512], BF16)
    nc.vector.memset(warm[:], 0.0)
    # lhsT per k: block-diagonal [[W_k, 0], [0, W_k]] so one 128-contraction
    # matmul computes tap k for BOTH image halves in a single rhs stream
    # (the 64-deep quadrant split would stream every pixel chunk twice)
    wT = inp.tile([128, 9, 128], BF16)
    bias = inp.tile([128, 9], F32)
    # x staged in 4 row-bands (16 rows + halo each) so the first matmuls can
    # start after ~1/4 of the input DMA; the eviction-critical xA copy is
    # issued before xB (evictions need all 4 matmul chunks = the full xA
    # band, while the muls that need xB trail the evictions anyway).
    xAb = [inp.tile([128, 18, 130], BF16, name=f"xAb{b}", tag=f"xA{b}")
           for b in range(4)]
    xBb = [inp.tile([128, 18, 128], BF16, name=f"xBb{b}", tag=f"xB{b}")
           for b in range(4)]
    nc.sync.dma_start(wT[:], wT_d[:])
    nc.sync.dma_start(xAb[0][:], xA_d[:, 0:18, :])
    nc.sync.dma_start(bias[:], bias_d[:])
    nc.sync.dma_start(xBb[0][:], xB_d[:, 0:18, :])
    for b in range(1, 4):
        nc.sync.dma_start(xAb[b][:], xA_d[:, 16 * b: 16 * b + 18, :])
        nc.sync.dma_start(xBb[b][:], xB_d[:, 16 * b: 16 * b + 18, :])

    arch = o.get("arch", "tree")
    psum = ctx.enter_context(tc.tile_pool(name="psum", bufs=o["psum_bufs"],
                                          space="PSUM"))
    fpool = ctx.enter_context(tc.tile_pool(name="f", bufs=o["fbufs"]))
    ppool = ctx.enter_context(tc.tile_pool(name="prod", bufs=o["pbufs"]))
    if arch == "fold":
        # out accumulates in PSUM via identity matmuls: no add tree at all
        accp = ctx.enter_context(tc.tile_pool(name="acc",
                                              bufs=o.get("accbufs", 1),
                                              space="PSUM"))
        opool = ctx.enter_context(tc.tile_pool(name="osb", bufs=2))
        ident = inp.tile([128, 128], BF16)
        nc.sync.dma_start(ident[:], ident_d[:])

    if o.get("warm", 8):
        wps = psum.tile([128, rt, 128], F32, name="warm_ps", tag="ps")
        for i in range(o.get("warm", 8)):
            nc.tensor.matmul(wps[0:64, 0:4, :], warm[0:64, 0:64],
                             warm[0:64].rearrange("p (a b) -> p a b", a=4),
                             start=True, stop=True)

    out4 = out_d.rearrange("c (h r) w -> h c r w", h=2)

    def _taps(v, default):
        if v is None:
            return default
        if isinstance(v, int):
            return (v,)
        if isinstance(v, str):
            return tuple(int(c) for c in v if c.isdigit())
        return tuple(v)

    stt_taps = _taps(o.get("stt"), ())
    dve_ev = _taps(o.get("dve_ev"), ())
    ms = o.get("msplit", 13)       # DVE rows per mul op; Pool gets the rest
    asp = o.get("asplit", 13)      # DVE rows per add op; Pool gets the rest
    if stt_taps:
        ones = inp.tile([1, 4, 128], BF16)
        nc.vector.memset(ones[:], 1.0)
        biasrow = inp.tile([1, 9, 128], BF16)
        nc.sync.dma_start(biasrow[:], biasrow_d[:])

    for t in range(nt * nreps):
        r0 = (t % nt) * rt
        band = r0 // 16
        lr0 = r0 - 16 * band
        xA, xB = xAb[band], xBb[band]
        f = fpool.tile([128, 9, rt, 128], BF16)
        prod = ppool.tile([128, 9, rt, 128], BF16)
        pss = {}

        def patch_ap(k, r_a, r_b):
            di, dj = divmod(k, 3)
            if dj == 1:
                return xB[:, lr0 + di + r_a: lr0 + di + r_b, 0:128]
            return xA[:, lr0 + di + r_a: lr0 + di + r_b, dj:dj + 128]

        def kgroup(k):
            ps = psum.tile([128, rt, 128], F32)
            pss[k] = ps
            if o.get("stage") != "nomm":
                fused_bias = k in stt_taps
                if fused_bias:
                    # bias arrives in PSUM via a K=1 ones matmul so the
                    # STT eviction-mul can fuse relu+mul without a bias op
                    for n in range(nch):
                        nc.tensor.matmul(ps[:, 4 * n: 4 * n + 4, :],
                                         biasrow[0:1, k, :], ones[0:1],
                                         start=True, stop=False)
                # center pixels: buffer row r+1, buffer cols 1..128
                for n in range(nch):  # <=512 fp32 cols per matmul (1 bank)
                    rhs = xA[:, lr0 + 1 + 4 * n: lr0 + 5 + 4 * n, 1:129]
                    nc.tensor.matmul(
                        ps[:, 4 * n: 4 * n + 4, :],
                        wT[:, k, :],
                        rhs,
                        start=not fused_bias, stop=True,
                    )
            if o.get("stage") == "noact" or k in stt_taps:
                return
            if k in dve_ev:
                nc.vector.tensor_scalar(f[:, k], ps[:], bias[:, k:k + 1], 0.0,
                                        op0=ADD, op1=MAX)
            else:
                nc.scalar.activation(f[:, k], ps[:], RELU, bias=bias[:, k:k + 1])

        def mul(k):
            if k in stt_taps:
                # fused relu+mul straight from PSUM (bias already inside)
                nc.vector.scalar_tensor_tensor(prod[:, k], pss[k][:], 0.0,
                                               patch_ap(k, 0, rt),
                                               op0=MAX, op1=MULT)
                return
            nc.vector.tensor_tensor(prod[:, k, 0:ms], f[:, k, 0:ms],
                                    patch_ap(k, 0, ms), op=MULT)
            if ms < rt:
                nc.gpsimd.tensor_tensor(prod[:, k, ms:rt], f[:, k, ms:rt],
                                        patch_ap(k, ms, rt), op=MULT)

        def add(dst, src):
            nc.vector.tensor_tensor(prod[:, dst, 0:asp], prod[:, src, 0:asp],
                                    prod[:, dst, 0:asp], op=ADD)
            if asp < rt:
                nc.gpsimd.tensor_tensor(prod[:, dst, asp:rt],
                                        prod[:, src, asp:rt],
                                        prod[:, dst, asp:rt], op=ADD)

        stage = o.get("stage", "full")
        if stage == "nomm":
            # pure ACT pace probe: all evictions read one pre-filled psum tile
            if t == 0:
                pse = psum.tile([128, rt, 128], F32, tag="ps")
                for n in range(nch):
                    nc.tensor.matmul(pse[:, 4 * n: 4 * n + 4, :],
                                     wT[:, 0, :],
                                     xA[:, 1 + 4 * n: 5 + 4 * n, 1:129],
                                     start=True, stop=True)
                o["_pse"] = pse
            for k in range(9):
                nc.scalar.activation(f[:, k], o["_pse"][:], RELU,
                                     bias=bias[:, k:k + 1])
            nc.sync.dma_start(out4[:, :, r0:r0 + rt, :], f[:, 8])
            continue
        if stage == "noact":
            # pure PE pace probe: matmuls only, DMA from the input band
            for k in range(9):
                kgroup(k)
            nc.sync.dma_start(out_d[:, r0:r0 + rt, :],
                              xA[0:64, 1:1 + rt, 1:129])
            continue
        if stage == "empty":
            # loop-overhead floor: input DMAs + output DMAs only
            nc.sync.dma_start(out_d[:, r0:r0 + rt, :],
                              xA[0:64, 1:1 + rt, 1:129])
            continue
        if stage == "evict":
            for k in range(9):
                kgroup(k)
            nc.sync.dma_start(out4[:, :, r0:r0 + rt, :], f[:, 8])
            continue
        if stage == "noadd":
            for k in range(9):
                kgroup(k)
            for k in range(9):
                mul(k)
            nc.sync.dma_start(out4[:, :, r0:r0 + rt, :], prod[:, 8])
            continue

        if arch == "fold":
            # every tap folds into a PSUM accumulator via identity matmuls;
            # no add tree, final copy PSUM -> SBUF bf16 on ACT, then DMA
            acc = accp.tile([128, rt, 128], F32)
            outsb = opool.tile([128, rt, 128], BF16)
            kseq = o.get("kseq", [0, 3, 1, 5, 2, 7, 4, 6, 8])
            fd = o.get("fdelay", 2)   # folds trail by fd taps so the PE's
                                      # in-order queue never head-of-line
                                      # blocks on the ACT->DVE chain

            def fold(j, k):
                for n in range(nch):
                    nc.tensor.matmul(
                        acc[:, 4 * n: 4 * n + 4, :], ident[:],
                        prod[:, k, 4 * n: 4 * n + 4],
                        start=(j == 0), stop=(j == len(kseq) - 1),
                    )

            for j, k in enumerate(kseq):
                kgroup(k)
                mul(k)
                if j >= fd:
                    fold(j - fd, kseq[j - fd])
            for j in range(len(kseq) - fd, len(kseq)):
                fold(j, kseq[j])
            nc.scalar.activation(outsb[:], acc[:],
                                 mybir.ActivationFunctionType.Copy)
            nc.sync.dma_start(out4[:, :, r0:r0 + rt, :], outsb[:])
            continue

        if o.get("chain", 1):
            # serial running-sum chain: every tap joins the accumulator at
            # depth 1, so after the LAST eviction only mul+add+DMA remain
            # (the balanced tree put the late taps 3 add-levels deep, which
            # serialized a ~9us drain after ACT's final eviction)
            for k in range(8):
                kgroup(k)
                mul(k)
                if k:
                    add(k, k - 1)
            kgroup(8)
            mul(8)
            add(8, 7)
            nc.sync.dma_start(out4[:, :, r0:r0 + rt, :], prod[:, 8])
            continue

        for k in range(8):
            kgroup(k)
        mul(0)
        mul(1)
        add(1, 0)
        mul(2)
        mul(3)
        add(3, 2)
        add(3, 1)
        mul(4)
        mul(5)
        add(5, 4)
        mul(6)
        mul(7)
        add(7, 6)
        add(7, 5)
        add(7, 3)

        kgroup(8)
        mul(8)
        if o.get("tail2", 0):
            # split the final add so the out DMA starts after the first half
            nc.vector.tensor_tensor(prod[:, 8, 0:8], prod[:, 7, 0:8],
                                    prod[:, 8, 0:8], op=ADD)
            nc.sync.dma_start(out4[:, :, r0:r0 + 8, :], prod[:, 8, 0:8])
            nc.vector.tensor_tensor(prod[:, 8, 8:asp], prod[:, 7, 8:asp],
                                    prod[:, 8, 8:asp], op=ADD)
            if asp < rt:
                nc.gpsimd.tensor_tensor(prod[:, 8, asp:rt], prod[:, 7, asp:rt],
                                        prod[:, 8, asp:rt], op=ADD)
            nc.sync.dma_start(out4[:, :, r0 + 8:r0 + rt, :], prod[:, 8, 8:rt])
        else:
            add(8, 7)
            nc.sync.dma_start(out4[:, :, r0:r0 + rt, :], prod[:, 8])


def _declare_tensors(nc):
    xA_d = nc.dram_tensor("xpadA", (128, 66, 130), BF16, kind="ExternalInput").ap()
    xB_d = nc.dram_tensor("xpadB", (128, 66, 128), BF16, kind="ExternalInput").ap()
    wT_d = nc.dram_tensor("wT", (128, 9, 128), BF16, kind="ExternalInput").ap()
    bias_d = nc.dram_tensor("bias", (128, 9), F32, kind="ExternalInput").ap()
    biasrow_d = nc.dram_tensor("biasrow", (1, 9, 128), BF16,
                               kind="ExternalInput").ap()
    ident_d = nc.dram_tensor("ident", (128, 128), BF16,
                             kind="ExternalInput").ap()
    out_d = nc.dram_tensor("out", (C, H, W), BF16, kind="ExternalOutput").ap()
    return out_d, xA_d, xB_d, wT_d, bias_d, biasrow_d, ident_d


def _build():
    if "nc" in _CACHE:
        return _CACHE["nc"]
    nc = bacc.Bacc("TRN2", target_bir_lowering=False, debug=False,
                   num_devices=NCORES)
    aps = _declare_tensors(nc)
    with tile.TileContext(nc) as tc, ExitStack() as ctx:
        _kernel_body(ctx, tc, *aps)
    nc.compile()
    _CACHE["nc"] = nc
    return nc


def _prep_core_inputs(x_i: np.ndarray, wT_np, bias_np, biasrow_np):
    """x_i: (C, H, W) float32 -> per-core input dict."""
    bf = ml_dtypes.bfloat16
    xA = np.zeros((128, 66, 130), dtype=bf)
    xB = np.zeros((128, 66, 128), dtype=bf)
    xb = x_i.astype(bf)
    # half A: buffer rows 0..65 = x rows -1..64 (row -1 zero-padded)
    xA[0:64, 1:66, 1:129] = xb[:, 0:65, :]
    xB[0:64, 1:66, :] = xb[:, 0:65, :]
    # half B: buffer rows 0..65 = x rows 63..128 (row 128 zero-padded)
    xA[64:128, 0:65, 1:129] = xb[:, 63:128, :]
    xB[64:128, 0:65, :] = xb[:, 63:128, :]
    return {"xpadA": xA, "xpadB": xB, "wT": wT_np, "bias": bias_np,
            "biasrow": biasrow_np, "ident": np.eye(128, dtype=bf)}


def _prep_inputs(x, W_gen, b_gen):
    x = np.asarray(x, dtype=np.float32)
    W_gen = np.asarray(W_gen, dtype=np.float32)
    b_gen = np.asarray(b_gen, dtype=np.float32)

    bf = ml_dtypes.bfloat16
    # lhsT: (c_in, k, c_out); o index in reference = c_out*9 + k.
    # Block-diagonal on (cin, cout) so one matmul serves both image halves.
    wT_half = W_gen.reshape(C, K * K, C).transpose(2, 1, 0).astype(bf)  # (cin,k,cout)
    wT_np = np.zeros((128, K * K, 128), dtype=bf)
    wT_np[0:C, :, 0:C] = wT_half
    wT_np[C:128, :, C:128] = wT_half
    b2 = b_gen.reshape(C, K * K).astype(np.float32)                     # (c_out, k)
    bias_np = np.ascontiguousarray(np.concatenate([b2, b2], axis=0))    # (128, 9)
    # per-k bias over the 128 (c, half) output channels, as a K=1 lhsT row
    biasrow_np = np.ascontiguousarray(
        bias_np.T.reshape(1, K * K, 128)).astype(bf)                    # (1, 9, 128)

    return [_prep_core_inputs(x[i], wT_np, bias_np, biasrow_np)
            for i in range(x.shape[0])]


def kernel(x: np.ndarray, W_gen: np.ndarray, b_gen: np.ndarray) -> np.ndarray:
    nc = _build()
    in_maps = _prep_inputs(x, W_gen, b_gen)
    res = run_bass_kernel_spmd(nc, in_maps, core_ids=list(range(NCORES)))
    out = np.stack([res.results[i]["out"] for i in range(NCORES)], axis=0)
    return out.astype(np.float32)


if __name__ == "__main__":
    xs = np.random.randn(B, C, H, W).astype(np.float32)
    Wg = np.random.randn(C * K * K, C).astype(np.float32) / np.sqrt(C)
    bg = (np.random.randn(C * K * K) * 0.01).astype(np.float32)
    o = kernel(xs, Wg, bg)
    print("out", o.shape, o.dtype, float(np.abs(o).mean()))



# revision 44
# speedup vs baseline: 1.2013x; 1.0074x over previous
"""ACDA (adaptive conv) Trainium2 kernel — 8-core data-parallel over batch.

Per core: one sample (C=64, H=128, W=128). The image is split into two
64-row halves stacked on the 128 SBUF partitions (partition p<64 -> half A
channel p, p>=64 -> half B channel p-64), so every engine op runs with all
128 lanes active.

The 1x1-conv filter generation runs as ONE 128-deep matmul per tap and
pixel chunk: the lhsT is block-diagonal [[W_k, 0], [0, W_k]] over
(c_in, half) x (c_out, half), so a single 512-column rhs stream produces
g_k for BOTH halves at once (the previous per-half 64-deep quadrant split
streamed every pixel chunk twice — 2x the PE time for the same output).

Host-side prep (inside kernel(), not on device): x is zero-padded, bf16-cast
and laid out per-core as two copies (xpadA with a left pad column, xpadB
column-shifted by one) so that all nine 3x3-tap shifts are 4-byte-aligned
SBUF views; weights are block-diagonal lhsT (c_in|half, k, c_out|half).

Pipeline per 16-row tile (both halves at once), overlapped by the Tile
scheduler; the Activation engine is the pacer (saturated end to end):
  DMA: padded x arrives in 4 row-bands (1 tile + halo each)
  PE:  g_k = [[W_k,0],[0,W_k]]^T @ x  (bf16 in, fp32 PSUM), 4 chunks per k
  ACT: f_k = relu(g_k + b_k)  PSUM -> SBUF bf16  (bias is per-partition)
  DVE: prod = f * patch rows 0:13; GPSIMD rows 13:16 (two independent
       streams sized so both fit under the ACT eviction pace)
  DVE/GP: serial running-sum chain prod[k] += prod[k-1] (same 13/3 row
       split): each tap joins the accumulator at depth 1, so after the
       LAST eviction only mul8 + one add + the out DMA remain (a balanced
       tree put late taps 3 add-levels deep and serialized a ~9us drain)
  DMA: prod[:,8] tile -> DRAM in bf16 (widened to fp32 on the host)

Alternative emission paths kept behind opts for experiments: arch="fold"
(PSUM accumulation of taps via identity matmuls), stt taps (bias preloaded
into PSUM by a K=1 ones matmul + fused relu-mul scalar_tensor_tensor on
DVE), dve_ev taps (eviction via DVE tensor_scalar), stage probes.
"""

import numpy as np
import ml_dtypes
from contextlib import ExitStack

import concourse.bass as bass
import concourse.tile as tile
from concourse import bacc, mybir
from concourse.bass_utils import run_bass_kernel_spmd

B, C, H, W, K = 8, 64, 128, 128, 3
NCORES = 8
RT = 16           # output rows per tile (per half)
BF16 = mybir.dt.bfloat16
F32 = mybir.dt.float32
RELU = mybir.ActivationFunctionType.Relu
MULT = mybir.AluOpType.mult
ADD = mybir.AluOpType.add
MAX = mybir.AluOpType.max

_CACHE = {}


def _window3(ap, lr0, dj, rt):
    """Overlapping 3-row window AP: [128, 3(di), rt, 128] over a padded-x
    band tile [128, nrows, rowlen], starting at buffer row lr0, col dj."""
    a = ap.copy()
    v = a.ap
    row_stride = v[1][0]
    v[1] = [row_stride, 3]
    v[2] = [row_stride, rt]
    v.append([1, 128])
    a.offset = a.offset + lr0 * row_stride + dj
    return a


def _kernel_body(ctx: ExitStack, tc, out_d, xA_d, xB_d, wT_d, bias_d,
                 biasrow_d, ident_d, nreps=1, opts=None):
    nc = tc.nc
    o = dict(rt=RT, psum_bufs=2, fbufs=2, pbufs=2)
    o.update(opts or {})
    rt = o["rt"]
    nt = 64 // rt
    nch = rt // 4          # 4-row matmul chunks per k-tile (1 PSUM bank each)

    inp = ctx.enter_context(tc.tile_pool(name="inp", bufs=1))
    # dep-free dummy relu so the one-time ACT table load runs at t~0 instead
    # of gating the first real eviction
    scratch = inp.tile([128, 2], F32)
    nc.gpsimd.memset(scratch[:], 0.0)
    nc.scalar.activation(scratch[:, 0:1], scratch[:, 1:2], RELU, bias=0.0)
    # dep-free dummy matmuls so the PE p-state ramps to full clock while the
    # input DMA is still in flight (HAM needs ~3us of sustained PE activity)
    warm = inp.tile([128, 512], BF16)
    nc.vector.memset(warm[:], 0.0)
    # lhsT per k: block-diagonal [[W_k, 0], [0, W_k]] so one 128-contraction
    # matmul computes tap k for BOTH image halves in a single rhs stream
    # (the 64-deep quadrant split would stream every pixel chunk twice)
    wT = inp.tile([128, 9, 128], BF16)
    bias = inp.tile([128, 9], F32)
    # x staged in 4 row-bands (16 rows + halo each) so the first matmuls can
    # start after ~1/4 of the input DMA; the eviction-critical xA copy is
    # issued before xB (evictions need all 4 matmul chunks = the full xA
    # band, while the muls that need xB trail the evictions anyway).
    xAb = [inp.tile([128, 18, 130], BF16, name=f"xAb{b}", tag=f"xA{b}")
           for b in range(4)]
    xBb = [inp.tile([128, 18, 128], BF16, name=f"xBb{b}", tag=f"xB{b}")
           for b in range(4)]
    # xA band 0 first: its transfer is the long pole for the first matmul;
    # wT (sub-us) lands during it, bias before the first eviction needs it
    nc.sync.dma_start(xAb[0][:], xA_d[:, 0:18, :])
    nc.sync.dma_start(wT[:], wT_d[:])
    nc.sync.dma_start(bias[:], bias_d[:])
    nc.sync.dma_start(xBb[0][:], xB_d[:, 0:18, :])
    for b in range(1, 4):
        nc.sync.dma_start(xAb[b][:], xA_d[:, 16 * b: 16 * b + 18, :])
        nc.sync.dma_start(xBb[b][:], xB_d[:, 16 * b: 16 * b + 18, :])

    arch = o.get("arch", "tree")
    psum = ctx.enter_context(tc.tile_pool(name="psum", bufs=o["psum_bufs"],
                                          space="PSUM"))
    fpool = ctx.enter_context(tc.tile_pool(name="f", bufs=o["fbufs"]))
    ppool = ctx.enter_context(tc.tile_pool(name="prod", bufs=o["pbufs"]))
    if arch == "fold":
        # out accumulates in PSUM via identity matmuls: no add tree at all
        accp = ctx.enter_context(tc.tile_pool(name="acc",
                                              bufs=o.get("accbufs", 1),
                                              space="PSUM"))
        opool = ctx.enter_context(tc.tile_pool(name="osb", bufs=2))
        ident = inp.tile([128, 128], BF16)
        nc.sync.dma_start(ident[:], ident_d[:])

    if o.get("warm", 8):
        wps = psum.tile([128, rt, 128], F32, name="warm_ps", tag="ps")
        for i in range(o.get("warm", 8)):
            nc.tensor.matmul(wps[0:64, 0:4, :], warm[0:64, 0:64],
                             warm[0:64].rearrange("p (a b) -> p a b", a=4),
                             start=True, stop=True)

    out4 = out_d.rearrange("c (h r) w -> h c r w", h=2)

    def _taps(v, default):
        if v is None:
            return default
        if isinstance(v, int):
            return (v,)
        if isinstance(v, str):
            return tuple(int(c) for c in v if c.isdigit())
        return tuple(v)

    stt_taps = _taps(o.get("stt"), ())
    dve_ev = _taps(o.get("dve_ev"), ())
    ms = o.get("msplit", 13)       # DVE rows per mul op; Pool gets the rest
    asp = o.get("asplit", 13)      # DVE rows per add op; Pool gets the rest
    if stt_taps:
        ones = inp.tile([1, 4, 128], BF16)
        nc.vector.memset(ones[:], 1.0)
        biasrow = inp.tile([1, 9, 128], BF16)
        nc.sync.dma_start(biasrow[:], biasrow_d[:])

    for t in range(nt * nreps):
        r0 = (t % nt) * rt
        band = r0 // 16
        lr0 = r0 - 16 * band
        xA, xB = xAb[band], xBb[band]
        f = fpool.tile([128, 9, rt, 128], BF16)
        prod = ppool.tile([128, 9, rt, 128], BF16)
        pss = {}

        def patch_ap(k, r_a, r_b):
            di, dj = divmod(k, 3)
            if dj == 1:
                return xB[:, lr0 + di + r_a: lr0 + di + r_b, 0:128]
            return xA[:, lr0 + di + r_a: lr0 + di + r_b, dj:dj + 128]

        def kgroup(k, evict=True):
            ps = psum.tile([128, rt, 128], F32)
            pss[k] = ps
            if o.get("stage") != "nomm":
                fused_bias = k in stt_taps
                if fused_bias:
                    # bias arrives in PSUM via a K=1 ones matmul so the
                    # STT eviction-mul can fuse relu+mul without a bias op
                    for n in range(nch):
                        nc.tensor.matmul(ps[:, 4 * n: 4 * n + 4, :],
                                         biasrow[0:1, k, :], ones[0:1],
                                         start=True, stop=False)
                # center pixels: buffer row r+1, buffer cols 1..128
                for n in range(nch):  # <=512 fp32 cols per matmul (1 bank)
                    rhs = xA[:, lr0 + 1 + 4 * n: lr0 + 5 + 4 * n, 1:129]
                    nc.tensor.matmul(
                        ps[:, 4 * n: 4 * n + 4, :],
                        wT[:, k, :],
                        rhs,
                        start=not fused_bias, stop=True,
                    )
            if o.get("stage") == "noact" or k in stt_taps or not evict:
                return
            if k in dve_ev:
                nc.vector.tensor_scalar(f[:, k], ps[:], bias[:, k:k + 1], 0.0,
                                        op0=ADD, op1=MAX)
            else:
                nc.scalar.activation(f[:, k], ps[:], RELU, bias=bias[:, k:k + 1])

        def mul(k):
            if k in stt_taps:
                # fused relu+mul straight from PSUM (bias already inside)
                nc.vector.scalar_tensor_tensor(prod[:, k], pss[k][:], 0.0,
                                               patch_ap(k, 0, rt),
                                               op0=MAX, op1=MULT)
                return
            nc.vector.tensor_tensor(prod[:, k, 0:ms], f[:, k, 0:ms],
                                    patch_ap(k, 0, ms), op=MULT)
            if ms < rt:
                nc.gpsimd.tensor_tensor(prod[:, k, ms:rt], f[:, k, ms:rt],
                                        patch_ap(k, ms, rt), op=MULT)

        def add(dst, src):
            nc.vector.tensor_tensor(prod[:, dst, 0:asp], prod[:, src, 0:asp],
                                    prod[:, dst, 0:asp], op=ADD)
            if asp < rt:
                nc.gpsimd.tensor_tensor(prod[:, dst, asp:rt],
                                        prod[:, src, asp:rt],
                                        prod[:, dst, asp:rt], op=ADD)

        stage = o.get("stage", "full")
        if stage == "nomm":
            # pure ACT pace probe: all evictions read one pre-filled psum tile
            if t == 0:
                pse = psum.tile([128, rt, 128], F32, tag="ps")
                for n in range(nch):
                    nc.tensor.matmul(pse[:, 4 * n: 4 * n + 4, :],
                                     wT[:, 0, :],
                                     xA[:, 1 + 4 * n: 5 + 4 * n, 1:129],
                                     start=True, stop=True)
                o["_pse"] = pse
            for k in range(9):
                nc.scalar.activation(f[:, k], o["_pse"][:], RELU,
                                     bias=bias[:, k:k + 1])
            nc.sync.dma_start(out4[:, :, r0:r0 + rt, :], f[:, 8])
            continue
        if stage == "noact":
            # pure PE pace probe: matmuls only, DMA from the input band
            for k in range(9):
                kgroup(k)
            nc.sync.dma_start(out_d[:, r0:r0 + rt, :],
                              xA[0:64, 1:1 + rt, 1:129])
            continue
        if stage == "empty":
            # loop-overhead floor: input DMAs + output DMAs only
            nc.sync.dma_start(out_d[:, r0:r0 + rt, :],
                              xA[0:64, 1:1 + rt, 1:129])
            continue
        if stage == "evict":
            for k in range(9):
                kgroup(k)
            nc.sync.dma_start(out4[:, :, r0:r0 + rt, :], f[:, 8])
            continue
        if stage == "noadd":
            for k in range(9):
                kgroup(k)
            for k in range(9):
                mul(k)
            nc.sync.dma_start(out4[:, :, r0:r0 + rt, :], prod[:, 8])
            continue

        if arch == "fold":
            # every tap folds into a PSUM accumulator via identity matmuls;
            # no add tree, final copy PSUM -> SBUF bf16 on ACT, then DMA
            acc = accp.tile([128, rt, 128], F32)
            outsb = opool.tile([128, rt, 128], BF16)
            kseq = o.get("kseq", [0, 3, 1, 5, 2, 7, 4, 6, 8])
            fd = o.get("fdelay", 2)   # folds trail by fd taps so the PE's
                                      # in-order queue never head-of-line
                                      # blocks on the ACT->DVE chain

            def fold(j, k):
                for n in range(nch):
                    nc.tensor.matmul(
                        acc[:, 4 * n: 4 * n + 4, :], ident[:],
                        prod[:, k, 4 * n: 4 * n + 4],
                        start=(j == 0), stop=(j == len(kseq) - 1),
                    )

            for j, k in enumerate(kseq):
                kgroup(k)
                mul(k)
                if j >= fd:
                    fold(j - fd, kseq[j - fd])
            for j in range(len(kseq) - fd, len(kseq)):
                fold(j, kseq[j])
            nc.scalar.activation(outsb[:], acc[:],
                                 mybir.ActivationFunctionType.Copy)
            nc.sync.dma_start(out4[:, :, r0:r0 + rt, :], outsb[:])
            continue

        if o.get("chain", 1):
            # serial running-sum chain: every tap joins the accumulator at
            # depth 1, so after the LAST eviction only mul+add+DMA remain
            # (the balanced tree put the late taps 3 add-levels deep, which
            # serialized a ~9us drain after ACT's final eviction)
            if o.get("k8first", 0):
                # tap 8 first: its PSUM tile is read by a fused STT deep in
                # DVE's queue; fronting it frees the buffer mid-tile instead
                # of lock-stepping the next tile's matmuls on psum rotation
                kgroup(8)
                mul(8)
            for k in range(8):
                kgroup(k)
                mul(k)
                if k:
                    add(k, k - 1)
            last = t == nt * nreps - 1
            if not o.get("k8first", 0):
                if last and o.get("tail3", 1):
                    # pipeline the kernel drain: split tap 8's eviction,
                    # mul, join-add and out DMA into row halves so the
                    # first half of the output is in flight while ACT
                    # still evicts the second half
                    kgroup(8, evict=False)
                    for (a, b) in ((0, 8), (8, rt)):
                        nc.scalar.activation(f[:, 8, a:b], pss[8][:, a:b],
                                             RELU, bias=bias[:, 8:9])
                        nc.vector.tensor_tensor(prod[:, 8, a:b], f[:, 8, a:b],
                                                patch_ap(8, a, b), op=MULT)
                        nc.vector.tensor_tensor(prod[:, 8, a:b],
                                                prod[:, 7, a:b],
                                                prod[:, 8, a:b], op=ADD)
                        nc.sync.dma_start(out4[:, :, r0 + a:r0 + b, :],
                                          prod[:, 8, a:b])
                    continue
                kgroup(8)
                mul(8)
            add(8, 7)
            nc.sync.dma_start(out4[:, :, r0:r0 + rt, :], prod[:, 8])
            continue

        for k in range(8):
            kgroup(k)
        mul(0)
        mul(1)
        add(1, 0)
        mul(2)
        mul(3)
        add(3, 2)
        add(3, 1)
        mul(4)
        mul(5)
        add(5, 4)
        mul(6)
        mul(7)
        add(7, 6)
        add(7, 5)
        add(7, 3)

        kgroup(8)
        mul(8)
        if o.get("tail2", 0):
            # split the final add so the out DMA starts after the first half
            nc.vector.tensor_tensor(prod[:, 8, 0:8], prod[:, 7, 0:8],
                                    prod[:, 8, 0:8], op=ADD)
            nc.sync.dma_start(out4[:, :, r0:r0 + 8, :], prod[:, 8, 0:8])
            nc.vector.tensor_tensor(prod[:, 8, 8:asp], prod[:, 7, 8:asp],
                                    prod[:, 8, 8:asp], op=ADD)
            if asp < rt:
                nc.gpsimd.tensor_tensor(prod[:, 8, asp:rt], prod[:, 7, asp:rt],
                                        prod[:, 8, asp:rt], op=ADD)
            nc.sync.dma_start(out4[:, :, r0 + 8:r0 + rt, :], prod[:, 8, 8:rt])
        else:
            add(8, 7)
            nc.sync.dma_start(out4[:, :, r0:r0 + rt, :], prod[:, 8])


def _declare_tensors(nc):
    xA_d = nc.dram_tensor("xpadA", (128, 66, 130), BF16, kind="ExternalInput").ap()
    xB_d = nc.dram_tensor("xpadB", (128, 66, 128), BF16, kind="ExternalInput").ap()
    wT_d = nc.dram_tensor("wT", (128, 9, 128), BF16, kind="ExternalInput").ap()
    bias_d = nc.dram_tensor("bias", (128, 9), F32, kind="ExternalInput").ap()
    biasrow_d = nc.dram_tensor("biasrow", (1, 9, 128), BF16,
                               kind="ExternalInput").ap()
    ident_d = nc.dram_tensor("ident", (128, 128), BF16,
                             kind="ExternalInput").ap()
    out_d = nc.dram_tensor("out", (C, H, W), BF16, kind="ExternalOutput").ap()
    return out_d, xA_d, xB_d, wT_d, bias_d, biasrow_d, ident_d


def _build():
    if "nc" in _CACHE:
        return _CACHE["nc"]
    nc = bacc.Bacc("TRN2", target_bir_lowering=False, debug=False,
                   num_devices=NCORES)
    aps = _declare_tensors(nc)
    with tile.TileContext(nc) as tc, ExitStack() as ctx:
        _kernel_body(ctx, tc, *aps)
    nc.compile()
    _CACHE["nc"] = nc
    return nc


def _prep_core_inputs(x_i: np.ndarray, wT_np, bias_np, biasrow_np):
    """x_i: (C, H, W) float32 -> per-core input dict."""
    bf = ml_dtypes.bfloat16
    xA = np.zeros((128, 66, 130), dtype=bf)
    xB = np.zeros((128, 66, 128), dtype=bf)
    xb = x_i.astype(bf)
    # half A: buffer rows 0..65 = x rows -1..64 (row -1 zero-padded)
    xA[0:64, 1:66, 1:129] = xb[:, 0:65, :]
    xB[0:64, 1:66, :] = xb[:, 0:65, :]
    # half B: buffer rows 0..65 = x rows 63..128 (row 128 zero-padded)
    xA[64:128, 0:65, 1:129] = xb[:, 63:128, :]
    xB[64:128, 0:65, :] = xb[:, 63:128, :]
    return {"xpadA": xA, "xpadB": xB, "wT": wT_np, "bias": bias_np,
            "biasrow": biasrow_np, "ident": np.eye(128, dtype=bf)}


def _prep_inputs(x, W_gen, b_gen):
    x = np.asarray(x, dtype=np.float32)
    W_gen = np.asarray(W_gen, dtype=np.float32)
    b_gen = np.asarray(b_gen, dtype=np.float32)

    bf = ml_dtypes.bfloat16
    # lhsT: (c_in, k, c_out); o index in reference = c_out*9 + k.
    # Block-diagonal on (cin, cout) so one matmul serves both image halves.
    wT_half = W_gen.reshape(C, K * K, C).transpose(2, 1, 0).astype(bf)  # (cin,k,cout)
    wT_np = np.zeros((128, K * K, 128), dtype=bf)
    wT_np[0:C, :, 0:C] = wT_half
    wT_np[C:128, :, C:128] = wT_half
    b2 = b_gen.reshape(C, K * K).astype(np.float32)                     # (c_out, k)
    bias_np = np.ascontiguousarray(np.concatenate([b2, b2], axis=0))    # (128, 9)
    # per-k bias over the 128 (c, half) output channels, as a K=1 lhsT row
    biasrow_np = np.ascontiguousarray(
        bias_np.T.reshape(1, K * K, 128)).astype(bf)                    # (1, 9, 128)

    return [_prep_core_inputs(x[i], wT_np, bias_np, biasrow_np)
            for i in range(x.shape[0])]


def kernel(x: np.ndarray, W_gen: np.ndarray, b_gen: np.ndarray) -> np.ndarray:
    nc = _build()
    in_maps = _prep_inputs(x, W_gen, b_gen)
    res = run_bass_kernel_spmd(nc, in_maps, core_ids=list(range(NCORES)))
    out = np.stack([res.results[i]["out"] for i in range(NCORES)], axis=0)
    return out.astype(np.float32)


if __name__ == "__main__":
    xs = np.random.randn(B, C, H, W).astype(np.float32)
    Wg = np.random.randn(C * K * K, C).astype(np.float32) / np.sqrt(C)
    bg = (np.random.randn(C * K * K) * 0.01).astype(np.float32)
    o = kernel(xs, Wg, bg)
    print("out", o.shape, o.dtype, float(np.abs(o).mean()))

